# revision 5
# baseline (speedup 1.0000x reference)
"""Trainium2 Bass kernel for nn_COS_Loss_45423574122758.

The reference crops (8,3,1024,1024) inputs to a 7x7 grid of 128x128
windows and computes per-window sums of x*t, x*x, t*t reduced over
batch+channel+window, then a cosine per window — but the final output
only reads cos[-1,-1]: the window at rows 768:896, cols 768:896. So the
scalar output depends only on the (8,3,128,128) last-window slice of
each input.

Strategy: shard that slice by batch across the 8 NeuronCores (one batch
per core). Each core DMAs its (3,128,128) slice pair viewed as
(128,384), computes per-partition partial sums of x*t, x*x, t*t with
three DVE tensor_tensor_reduce ops, and DMAs out a (128,3) stats tile.
The host sums the 8x128x3 partials and finishes the scalar cosine math.
"""

import numpy as np

import concourse.bass as bass
import concourse.tile as tile
from concourse import bacc, mybir
from concourse.bass_utils import run_bass_kernel_spmd

_K = 128          # sliding window size
_R0 = 768         # last window start: (ceil((1024-128)/128) - 1) * 128
_B = 8
_NPART = 128      # SBUF partitions
_NFREE = 384      # 3 channels * 128 cols per partition row
_COUNT = 49.0     # 7*7 windows

# Set by test.py to capture a neuron-profile trace; harness leaves it off.
PROFILE = False
LAST_EXEC_TIME_NS = None

_cached = {}


def _program() -> bass.Bass:
    if "nc" in _cached:
        return _cached["nc"]

    nc = bacc.Bacc(
        trn_type="TRN2",
        target_bir_lowering=False,
        debug=False,
        num_devices=_B,
    )
    x_d = nc.dram_tensor("x", [_NPART, _NFREE], mybir.dt.float32,
                         kind="ExternalInput").ap()
    t_d = nc.dram_tensor("t", [_NPART, _NFREE], mybir.dt.float32,
                         kind="ExternalInput").ap()
    s_d = nc.dram_tensor("stats", [_NPART, 3], mybir.dt.float32,
                         kind="ExternalOutput").ap()

    ax_x = mybir.AxisListType.X

    with tile.TileContext(nc) as tc:
        with tc.tile_pool(name="p", bufs=1) as pool:
            X = pool.tile([_NPART, _NFREE], mybir.dt.float32)
            T = pool.tile([_NPART, _NFREE], mybir.dt.float32)
            P = pool.tile([_NPART, _NFREE], mybir.dt.float32)
            S = pool.tile([_NPART, 3], mybir.dt.float32)
            nc.sync.dma_start(out=X[:], in_=x_d[:])
            nc.sync.dma_start(out=T[:], in_=t_d[:])
            nc.vector.tensor_mul(P[:], X[:], T[:])
            nc.vector.reduce_sum(S[:, 0:1], P[:], axis=ax_x)
            nc.vector.tensor_mul(P[:], X[:], X[:])
            nc.vector.reduce_sum(S[:, 1:2], P[:], axis=ax_x)
            nc.vector.tensor_mul(P[:], T[:], T[:])
            nc.vector.reduce_sum(S[:, 2:3], P[:], axis=ax_x)
            nc.sync.dma_start(out=s_d[:], in_=S[:])

    nc.compile()
    _cached["nc"] = nc
    return nc


def kernel(input: np.ndarray, target: np.ndarray) -> np.ndarray:
    global LAST_EXEC_TIME_NS
    inp = np.asarray(input, dtype=np.float32)
    tar = np.asarray(target, dtype=np.float32)

    xs = inp[:, :, _R0:_R0 + _K, _R0:_R0 + _K]  # (8,3,128,128)
    ts = tar[:, :, _R0:_R0 + _K, _R0:_R0 + _K]
    in_maps = [
        {
            "x": np.ascontiguousarray(xs[b]).reshape(_NPART, _NFREE),
            "t": np.ascontiguousarray(ts[b]).reshape(_NPART, _NFREE),
        }
        for b in range(_B)
    ]

    nc = _program()
    res = run_bass_kernel_spmd(nc, in_maps, core_ids=list(range(_B)),
                               trace=PROFILE)
    LAST_EXEC_TIME_NS = res.exec_time_ns

    stats = np.stack([res.results[b]["stats"] for b in range(_B)])  # (8,128,3)
    dot, ni, nt = stats.astype(np.float64).sum(axis=(0, 1))
    cos = dot / (np.sqrt(ni) * np.sqrt(nt))
    return np.array((cos - 1.0) ** 2 / _COUNT, dtype=np.float32)
